# revision 19
# baseline (speedup 1.0000x reference)
"""CostVolume2D Trainium2 kernel, v6.

out[b, i, h, w] = mean_c l[b,c,h,w] * r_pad[b,c,h, w + 96 - i]   (zero padded)

Design (H sharded over 8 cores, no halo):
  * 32-w blocks, 128-wide disparity windows: stationary = l[:, 32 w's],
    moving = 128-wide r window at r_pad col 32*blk. Out tile [32 w, 128 jj],
    jj = ww + 96 - i. Band overhead 128/97 = 1.32x.
  * Shared wide LDWEIGHTS: one [128, 128] load fills both PE row halves
    (2 h's) x 4 col tiles (4 w-blocks) = weights for 8 matmuls. The 8
    auto-generated narrow InstLdweights are deleted post-build, their
    sync_info merged into the matmuls (f16 supports non-self-loading
    InstMatmult).
  * No zero pads: edge blocks trim windows to real r columns; the host
    zeroes out-of-image cells (w + 96 - i outside [48, 560)).
  * lr tile [128, 2048] holds 4 h's (2 partition halves x 2 col blocks):
    one input DMA per 4 h, 2048B descriptors.
  * PSUM [128, 512] = one bank per (b,h): 16 matmuls (4 pg x 4 qg). One
    big eviction per h (DVE/Act alternate).
  * g tile [128, 4096] = 8 h's; one store DMA per 8 h, 8KB descriptors.
"""

import sys

try:
    import concourse  # noqa: F401
except ImportError:
    sys.path.insert(0, "/opt/trn_rl_repo")

import numpy as np

from concourse import bass, mybir
from concourse import tile
from concourse.ap import AP
from concourse.bass_utils import run_bass_kernel_spmd

F32 = mybir.dt.float32
F16 = mybir.dt.float16

# Problem dims (hardcoded per spec)
B, C, H, W = 4, 64, 256, 512
MAXD = 48
D = 2 * MAXD + 1          # 97 disparity planes
NCORES = 8
HS = H // NCORES          # 32 h-rows per core

MB = 32                   # stationary w-block
NBLK = W // MB            # 16 blocks per h row
JW = 128                  # disparity window width per block
RP0 = MAXD                # r_pad col of first real r column (48)
RP1 = RP0 + W             # r_pad col past last real r column (560)

PSW = 512                 # psum bank cols per (b,h): 4 groups x 128
HG = 2                    # h rows per g tile / store DMA
GW = HG * PSW             # 2048 g cols
EROW = 128 * GW           # 262144 slots per (b, hq)
LRW = 2 * 2 * W           # 2048 lr tile cols (2 h-pairs)

LAST_RESULTS = None
_NC_CACHE = {}


def _build_nc(b_n=B, hs=HS):
    nc = bass.Bass()
    # lr packed h-major, no pads: [b, h, c, 1024] = [l | r]
    lr_in = nc.dram_tensor("lr", [b_n, hs, C, 2 * W], F16,
                           kind="ExternalInput")
    o_out = nc.dram_tensor("o", [b_n, hs // HG, EROW], F16,
                           kind="ExternalOutput")

    keep_lds = set()
    with tile.TileContext(nc) as tc:
        with (
            tc.tile_pool(name="lrpool", bufs=10) as lrp,
            tc.tile_pool(name="gpool", bufs=6) as gp,
            tc.tile_pool(name="ppool", bufs=8, space="PSUM") as pp,
        ):
            for b in range(b_n):
                for hq in range(hs // HG):
                    g = gp.tile([128, GW], F16, name="g", tag="g")
                    for hp in range(HG // 2):
                        h0 = HG * hq + 2 * hp
                        lr_t = lrp.tile([128, 2 * W], F16, name="lr_t")
                        # one DMA per h-pair; 2048B descriptor per (h, c)
                        lr_src = AP(
                            lr_in, ((b * hs + h0) * C) * (2 * W),
                            [(C * 2 * W, 2), (2 * W, C), (1, 2 * W)],
                        )
                        lr_dst = AP(
                            lr_t.tensor, lr_t.offset,
                            [(lr_t.ap[0][0], 128), (1, 2 * W)],
                        )
                        nc.sync.dma_start(out=lr_dst, in_=lr_src)
                        pts = [pp.tile([128, PSW], F32, name="p_t")
                               for _ in range(2)]
                        for qg in range(4):
                            ld = nc.tensor.ldweights(
                                lr_t[0:128, JW * qg:JW * qg + JW],
                                tile_position=(0, 0),
                            )
                            keep_lds.add(ld.ins.name)
                            for e in range(2):
                                for pg in range(4):
                                    blk = 4 * qg + pg
                                    jlo = max(0, RP0 - MB * blk)
                                    jhi = min(JW, RP1 - MB * blk)
                                    r0 = W + MB * blk + jlo - RP0
                                    nc.tensor.matmul(
                                        pts[e][32 * pg:32 * pg + 32,
                                               JW * qg + jlo:JW * qg + jhi],
                                        lr_t[64 * e:64 * e + 64,
                                             MB * blk:MB * blk + MB],
                                        lr_t[64 * e:64 * e + 64,
                                             r0:r0 + jhi - jlo],
                                        start=True, stop=True,
                                        tile_position=(64 * e, 32 * pg),
                                    )
                        for e in range(2):
                            hm = 2 * hp + e
                            dst = g[:, PSW * hm:PSW * hm + PSW]
                            if hm % 2 == 0:
                                nc.vector.tensor_copy(dst, pts[e][:])
                            else:
                                nc.scalar.copy(dst, pts[e][:])
                    # one store per 8 h rows: per-partition contiguous 8KB.
                    # Kicked from gpsimd (SWDGE): its eviction-completion
                    # wait must not block the sync engine's input kicks.
                    d_ap = AP(o_out, (b * (hs // HG) + hq) * EROW,
                              [(GW, 128), (1, GW)])
                    nc.gpsimd.dma_start(out=d_ap, in_=g[:, :])
    _drop_narrow_ldweights(nc, keep_lds)
    _split_multi_waits(nc)
    return nc


def _drop_narrow_ldweights(nc, keep_lds):
    """Delete auto-generated narrow InstLdweights (each matmul self-loads by
    default); the explicit wide [128,128] loads already hold the weights.
    Each narrow LD's sync_info merges into its matmul (the next PE instr)."""
    for bb in nc.main_func.blocks:
        new_list = []
        pending = None
        changed = False
        for ins in bb.instructions:
            if isinstance(ins, mybir.InstLdweights) and ins.name not in keep_lds:
                assert pending is None, "two narrow LDs without a matmul"
                pending = ins
                changed = True
                continue
            if pending is not None and isinstance(ins, mybir.InstMatmult):
                lsi = pending.sync_info
                msi = ins.sync_info
                waits = (list(lsi.on_wait) if lsi else []) + \
                        (list(msi.on_wait) if msi else [])
                updates = (list(lsi.on_update) if lsi else []) + \
                          (list(msi.on_update) if msi else [])
                if waits or updates:
                    ins.sync_info = mybir.SyncInfo(
                        on_wait=waits, on_update=updates)
                pending = None
            new_list.append(ins)
        assert pending is None
        if changed:
            bb.instructions = new_list


def _split_multi_waits(nc):
    """Hoist all but one semaphore wait onto standalone InstEventSemaphore
    instructions (64-byte TPB encoding holds a single wait)."""
    for bb in nc.main_func.blocks:
        new_list = []
        changed = False
        for ins in bb.instructions:
            si = ins.sync_info
            if si is not None and len(si.on_wait) > 1:
                for w in list(si.on_wait)[:-1]:
                    ev = mybir.InstEventSemaphore(
                        name=nc.get_next_instruction_name(),
                        engine=ins.engine,
                        ins=[],
                        outs=[],
                        sync_info=mybir.SyncInfo(on_wait=[w], on_update=[]),
                    )
                    new_list.append(ev)
                ins.sync_info = mybir.SyncInfo(
                    on_wait=[list(si.on_wait)[-1]], on_update=list(si.on_update)
                )
                changed = True
            new_list.append(ins)
        if changed:
            bb.instructions = new_list


def _get_nc():
    key = (B, HS)
    if key not in _NC_CACHE:
        _NC_CACHE[key] = _build_nc()
    return _NC_CACHE[key]


def _host_prep(l_fmap, r_fmap):
    l = np.asarray(l_fmap, dtype=np.float32)
    r = np.asarray(r_fmap, dtype=np.float32)
    l = l * np.float32(1.0 / C)
    # pack [B, C, H, *] -> [B, H, C, 1024] = [l | r]
    lr = np.empty((B, H, C, 2 * W), dtype=np.float16)
    lr[..., :W] = l.transpose(0, 2, 1, 3)
    lr[..., W:] = r.transpose(0, 2, 1, 3)
    return lr


def _install_ntff_hook_shim(so_path="/opt/axon/libaxon_pjrt.so"):
    import types
    import ctypes
    import contextlib

    try:
        from antenv.axon_hooks import get_axon_ntff_profile_hook  # noqa: F401
        return
    except ImportError:
        pass

    lib = ctypes.CDLL(so_path)
    if not hasattr(lib, "axon_start_nrt_profile"):
        return
    lib.axon_start_nrt_profile.argtypes = [
        ctypes.POINTER(ctypes.c_int64), ctypes.c_size_t,
    ]
    lib.axon_start_nrt_profile.restype = ctypes.c_int64
    lib.axon_stop_nrt_profile.argtypes = [ctypes.c_char_p]
    lib.axon_stop_nrt_profile.restype = ctypes.c_int64

    @contextlib.contextmanager
    def _hook(output_dir, device_ids):
        import jax
        jax.devices()
        if device_ids:
            ids = (ctypes.c_int64 * len(device_ids))(*device_ids)
            rc = lib.axon_start_nrt_profile(ids, len(device_ids))
        else:
            rc = lib.axon_start_nrt_profile(None, 0)
        if rc != 0:
            raise RuntimeError(f"axon_start_nrt_profile rc={rc}")
        try:
            yield
        finally:
            n = lib.axon_stop_nrt_profile(str(output_dir).encode())
            print(f"ntff profile: {n} file(s) written to {output_dir}",
                  file=sys.stderr)

    import antenv
    mod = types.ModuleType("antenv.axon_hooks")
    mod.get_axon_ntff_profile_hook = lambda: _hook
    mod.set_axon_ntff_profile_hook = lambda h: None
    sys.modules["antenv.axon_hooks"] = mod
    antenv.axon_hooks = mod


def _unshard(res):
    # o[b, hq, p, hm, qg, jj]: h = 8hq + hm, w = 128qg + p (p = 32pg + ww),
    # value for plane i at jj = (p % 32) + 96 - i; zero when the r_pad
    # column 32blk + jj = w + 96 - i falls outside the real image [48, 560).
    out = np.empty((B, D, H, W), dtype=np.float32)
    pmod = np.arange(128) % 32                       # ww per partition
    idx = (pmod[:, None] + 96 - np.arange(D)[None, :])   # [128, 97]
    idx6 = np.broadcast_to(idx.reshape(1, 1, 128, 1, 1, D),
                           (B, HS // HG, 128, HG, NBLK // 4, D))
    # zero mask: w + 96 - i outside [48, 560)
    wv = np.arange(W)[None, :]
    iv = np.arange(D)[:, None]
    rcol = wv + 96 - iv                              # [97, 512]
    zmask = (rcol < RP0) | (rcol >= RP1)             # [97, 512]
    for k in range(NCORES):
        o = np.asarray(res.results[k]["o"])          # [B, 4, 524288] f16
        o6 = o.reshape(B, HS // HG, 128, HG, NBLK // 4, JW)
        v = np.take_along_axis(o6, idx6, axis=5)     # [B, hq, p, hm, qg, 97]
        # -> [B, i, hq, hm, qg, p] -> [B, D, HS, W] (w = 128qg + p)
        t = v.transpose(0, 5, 1, 3, 4, 2).astype(np.float32)
        blkout = t.reshape(B, D, HS, W)
        blkout[:, :, :, :] = np.where(zmask[None, :, None, :], 0.0, blkout)
        out[:, :, k * HS:(k + 1) * HS, :] = blkout
    return out


def kernel(l_fmap, r_fmap, max_disp):
    global LAST_RESULTS
    assert int(max_disp) == MAXD
    lr = _host_prep(l_fmap, r_fmap)

    nc = _get_nc()
    in_maps = []
    for k in range(NCORES):
        sl = slice(k * HS, (k + 1) * HS)
        in_maps.append({"lr": np.ascontiguousarray(lr[:, sl])})

    import os
    trace = bool(int(os.environ.get("CV_TRACE", "0")))
    if trace:
        _install_ntff_hook_shim()
    res = run_bass_kernel_spmd(nc, in_maps, list(range(NCORES)), trace=trace)
    LAST_RESULTS = res
    return _unshard(res)


# revision 20
# speedup vs baseline: 1.0553x; 1.0553x over previous
"""CostVolume2D Trainium2 kernel, v10 (best: ~110.5us, from 158.5us baseline).

out[b, i, h, w] = mean_c l[b,c,h,w] * r_pad[b,c,h, w + 96 - i]   (zero padded)

Design (H sharded over 8 cores, no halo):
  * 32-w blocks, 128-wide disparity windows: stationary = l[:, 32 w's],
    moving = 128-wide r window at r_pad col 32*blk. Out tile [32 w, 128 jj],
    jj = ww + 96 - i. Band overhead 128/97 = 1.32x.
  * Shared wide LDWEIGHTS: one [128, 128] load fills both PE row halves
    (2 h's) x 4 col tiles (4 w-blocks) = weights for 8 matmuls. The 8
    auto-generated narrow InstLdweights are deleted post-build, their
    sync_info merged into the matmuls (f16 supports non-self-loading
    InstMatmult).
  * No zero pads: edge blocks trim windows to real r columns; the host
    zeroes out-of-image cells (w + 96 - i outside [48, 560)).
  * lr tile [128, 1024] = one h-pair on partition halves; one input DMA
    per pair (sync-kicked), fully contiguous 2048B descriptors.
  * PSUM [128, 512] = one bank per (b,h): 16 matmuls (4 pg x 4 qg). One
    big eviction per h (DVE/Act alternate).
  * g tile [128, 2048] = 4 h's; one store DMA per 4 h, 4KB per-partition
    contiguous descriptors, kicked from gpsimd (SWDGE) so its
    eviction-completion wait cannot block sync's input kicks.
"""

import sys

try:
    import concourse  # noqa: F401
except ImportError:
    sys.path.insert(0, "/opt/trn_rl_repo")

import numpy as np

from concourse import bass, mybir
from concourse import tile
from concourse.ap import AP
from concourse.bass_utils import run_bass_kernel_spmd

F32 = mybir.dt.float32
F16 = mybir.dt.float16

# Problem dims (hardcoded per spec)
B, C, H, W = 4, 64, 256, 512
MAXD = 48
D = 2 * MAXD + 1          # 97 disparity planes
NCORES = 8
HS = H // NCORES          # 32 h-rows per core

MB = 32                   # stationary w-block
NBLK = W // MB            # 16 blocks per h row
JW = 128                  # disparity window width per block
RP0 = MAXD                # r_pad col of first real r column (48)
RP1 = RP0 + W             # r_pad col past last real r column (560)

PSW = 512                 # psum bank cols per (b,h): 4 groups x 128
HG = 4                    # h rows per g tile / store DMA
GW = HG * PSW             # 2048 g cols
EROW = 128 * GW           # 262144 slots per (b, hq)

LAST_RESULTS = None
_NC_CACHE = {}


def _build_nc(b_n=B, hs=HS):
    nc = bass.Bass()
    # lr packed h-major, no pads: [b, h, c, 1024] = [l | r]
    lr_in = nc.dram_tensor("lr", [b_n, hs, C, 2 * W], F16,
                           kind="ExternalInput")
    o_out = nc.dram_tensor("o", [b_n, hs // HG, EROW], F16,
                           kind="ExternalOutput")

    keep_lds = set()
    with tile.TileContext(nc) as tc:
        with (
            tc.tile_pool(name="lrpool", bufs=10) as lrp,
            tc.tile_pool(name="gpool", bufs=6) as gp,
            tc.tile_pool(name="ppool", bufs=8, space="PSUM") as pp,
        ):
            for b in range(b_n):
                for hq in range(hs // HG):
                    g = gp.tile([128, GW], F16, name="g", tag="g")
                    for hp in range(HG // 2):
                        h0 = HG * hq + 2 * hp
                        lr_t = lrp.tile([128, 2 * W], F16, name="lr_t")
                        # one DMA per h-pair; 2048B descriptor per (h, c)
                        lr_src = AP(
                            lr_in, ((b * hs + h0) * C) * (2 * W),
                            [(C * 2 * W, 2), (2 * W, C), (1, 2 * W)],
                        )
                        lr_dst = AP(
                            lr_t.tensor, lr_t.offset,
                            [(lr_t.ap[0][0], 128), (1, 2 * W)],
                        )
                        nc.sync.dma_start(out=lr_dst, in_=lr_src)
                        pts = [pp.tile([128, PSW], F32, name="p_t")
                               for _ in range(2)]
                        for qg in range(4):
                            ld = nc.tensor.ldweights(
                                lr_t[0:128, JW * qg:JW * qg + JW],
                                tile_position=(0, 0),
                            )
                            keep_lds.add(ld.ins.name)
                            for e in range(2):
                                for pg in range(4):
                                    blk = 4 * qg + pg
                                    jlo = max(0, RP0 - MB * blk)
                                    jhi = min(JW, RP1 - MB * blk)
                                    r0 = W + MB * blk + jlo - RP0
                                    nc.tensor.matmul(
                                        pts[e][32 * pg:32 * pg + 32,
                                               JW * qg + jlo:JW * qg + jhi],
                                        lr_t[64 * e:64 * e + 64,
                                             MB * blk:MB * blk + MB],
                                        lr_t[64 * e:64 * e + 64,
                                             r0:r0 + jhi - jlo],
                                        start=True, stop=True,
                                        tile_position=(64 * e, 32 * pg),
                                    )
                        for e in range(2):
                            hm = 2 * hp + e
                            dst = g[:, PSW * hm:PSW * hm + PSW]
                            if hm % 2 == 0:
                                nc.vector.tensor_copy(dst, pts[e][:])
                            else:
                                nc.scalar.copy(dst, pts[e][:])
                    # one store per 8 h rows: per-partition contiguous 8KB.
                    # Kicked from gpsimd (SWDGE): its eviction-completion
                    # wait must not block the sync engine's input kicks.
                    d_ap = AP(o_out, (b * (hs // HG) + hq) * EROW,
                              [(GW, 128), (1, GW)])
                    nc.gpsimd.dma_start(out=d_ap, in_=g[:, :])
    _drop_narrow_ldweights(nc, keep_lds)
    _split_multi_waits(nc)
    return nc


def _drop_narrow_ldweights(nc, keep_lds):
    """Delete auto-generated narrow InstLdweights (each matmul self-loads by
    default); the explicit wide [128,128] loads already hold the weights.
    Each narrow LD's sync_info merges into its matmul (the next PE instr)."""
    for bb in nc.main_func.blocks:
        new_list = []
        pending = None
        changed = False
        for ins in bb.instructions:
            if isinstance(ins, mybir.InstLdweights) and ins.name not in keep_lds:
                assert pending is None, "two narrow LDs without a matmul"
                pending = ins
                changed = True
                continue
            if pending is not None and isinstance(ins, mybir.InstMatmult):
                lsi = pending.sync_info
                msi = ins.sync_info
                waits = (list(lsi.on_wait) if lsi else []) + \
                        (list(msi.on_wait) if msi else [])
                updates = (list(lsi.on_update) if lsi else []) + \
                          (list(msi.on_update) if msi else [])
                if waits or updates:
                    ins.sync_info = mybir.SyncInfo(
                        on_wait=waits, on_update=updates)
                pending = None
            new_list.append(ins)
        assert pending is None
        if changed:
            bb.instructions = new_list


def _split_multi_waits(nc):
    """Hoist all but one semaphore wait onto standalone InstEventSemaphore
    instructions (64-byte TPB encoding holds a single wait)."""
    for bb in nc.main_func.blocks:
        new_list = []
        changed = False
        for ins in bb.instructions:
            si = ins.sync_info
            if si is not None and len(si.on_wait) > 1:
                for w in list(si.on_wait)[:-1]:
                    ev = mybir.InstEventSemaphore(
                        name=nc.get_next_instruction_name(),
                        engine=ins.engine,
                        ins=[],
                        outs=[],
                        sync_info=mybir.SyncInfo(on_wait=[w], on_update=[]),
                    )
                    new_list.append(ev)
                ins.sync_info = mybir.SyncInfo(
                    on_wait=[list(si.on_wait)[-1]], on_update=list(si.on_update)
                )
                changed = True
            new_list.append(ins)
        if changed:
            bb.instructions = new_list


def _get_nc():
    key = (B, HS)
    if key not in _NC_CACHE:
        _NC_CACHE[key] = _build_nc()
    return _NC_CACHE[key]


def _host_prep(l_fmap, r_fmap):
    l = np.asarray(l_fmap, dtype=np.float32)
    r = np.asarray(r_fmap, dtype=np.float32)
    l = l * np.float32(1.0 / C)
    # pack [B, C, H, *] -> [B, H, C, 1024] = [l | r]
    lr = np.empty((B, H, C, 2 * W), dtype=np.float16)
    lr[..., :W] = l.transpose(0, 2, 1, 3)
    lr[..., W:] = r.transpose(0, 2, 1, 3)
    return lr


def _install_ntff_hook_shim(so_path="/opt/axon/libaxon_pjrt.so"):
    import types
    import ctypes
    import contextlib

    try:
        from antenv.axon_hooks import get_axon_ntff_profile_hook  # noqa: F401
        return
    except ImportError:
        pass

    lib = ctypes.CDLL(so_path)
    if not hasattr(lib, "axon_start_nrt_profile"):
        return
    lib.axon_start_nrt_profile.argtypes = [
        ctypes.POINTER(ctypes.c_int64), ctypes.c_size_t,
    ]
    lib.axon_start_nrt_profile.restype = ctypes.c_int64
    lib.axon_stop_nrt_profile.argtypes = [ctypes.c_char_p]
    lib.axon_stop_nrt_profile.restype = ctypes.c_int64

    @contextlib.contextmanager
    def _hook(output_dir, device_ids):
        import jax
        jax.devices()
        if device_ids:
            ids = (ctypes.c_int64 * len(device_ids))(*device_ids)
            rc = lib.axon_start_nrt_profile(ids, len(device_ids))
        else:
            rc = lib.axon_start_nrt_profile(None, 0)
        if rc != 0:
            raise RuntimeError(f"axon_start_nrt_profile rc={rc}")
        try:
            yield
        finally:
            n = lib.axon_stop_nrt_profile(str(output_dir).encode())
            print(f"ntff profile: {n} file(s) written to {output_dir}",
                  file=sys.stderr)

    import antenv
    mod = types.ModuleType("antenv.axon_hooks")
    mod.get_axon_ntff_profile_hook = lambda: _hook
    mod.set_axon_ntff_profile_hook = lambda h: None
    sys.modules["antenv.axon_hooks"] = mod
    antenv.axon_hooks = mod


def _unshard(res):
    # o[b, hq, p, hm, qg, jj]: h = 8hq + hm, w = 128qg + p (p = 32pg + ww),
    # value for plane i at jj = (p % 32) + 96 - i; zero when the r_pad
    # column 32blk + jj = w + 96 - i falls outside the real image [48, 560).
    out = np.empty((B, D, H, W), dtype=np.float32)
    pmod = np.arange(128) % 32                       # ww per partition
    idx = (pmod[:, None] + 96 - np.arange(D)[None, :])   # [128, 97]
    idx6 = np.broadcast_to(idx.reshape(1, 1, 128, 1, 1, D),
                           (B, HS // HG, 128, HG, NBLK // 4, D))
    # zero mask: w + 96 - i outside [48, 560)
    wv = np.arange(W)[None, :]
    iv = np.arange(D)[:, None]
    rcol = wv + 96 - iv                              # [97, 512]
    zmask = (rcol < RP0) | (rcol >= RP1)             # [97, 512]
    for k in range(NCORES):
        o = np.asarray(res.results[k]["o"])          # [B, 4, 524288] f16
        o6 = o.reshape(B, HS // HG, 128, HG, NBLK // 4, JW)
        v = np.take_along_axis(o6, idx6, axis=5)     # [B, hq, p, hm, qg, 97]
        # -> [B, i, hq, hm, qg, p] -> [B, D, HS, W] (w = 128qg + p)
        t = v.transpose(0, 5, 1, 3, 4, 2).astype(np.float32)
        blkout = t.reshape(B, D, HS, W)
        blkout[:, :, :, :] = np.where(zmask[None, :, None, :], 0.0, blkout)
        out[:, :, k * HS:(k + 1) * HS, :] = blkout
    return out


def kernel(l_fmap, r_fmap, max_disp):
    global LAST_RESULTS
    assert int(max_disp) == MAXD
    lr = _host_prep(l_fmap, r_fmap)

    nc = _get_nc()
    in_maps = []
    for k in range(NCORES):
        sl = slice(k * HS, (k + 1) * HS)
        in_maps.append({"lr": np.ascontiguousarray(lr[:, sl])})

    import os
    trace = bool(int(os.environ.get("CV_TRACE", "0")))
    if trace:
        _install_ntff_hook_shim()
    res = run_bass_kernel_spmd(nc, in_maps, list(range(NCORES)), trace=trace)
    LAST_RESULTS = res
    return _unshard(res)
